# revision 4
# baseline (speedup 1.0000x reference)
"""Trainium2 Bass kernel for nn_DomainDiscriminator.

Network: conv(512->256,k3,s3,p1) -> BN -> conv(256->128,k3,s3,p1) -> BN
         -> reshape -> 12-layer MLP (3200->2048->...->1) -> sigmoid.
Input x: [64, 512, 40, 40] f32.  Output: [64, 1] f32.

Strategy (8 NeuronCores):
 - Data-parallel batch shard (8 per core) for the convs.
 - stride==kernel==3 convs are non-overlapping patch matmuls. Conv1 patches are
   built host-side (space-to-depth, free); conv2 patches are read straight out
   of SBUF with strided access patterns (boundary-split matmuls, no im2col).
 - Training-mode BN: conv bias is absorbed exactly by BN; BN1 stats via a 2KB
   AllReduce; BN2 stats computed redundantly after an AllGather of the conv2
   raw output (which the MLP needs anyway).
 - MLP: L3 column-parallel (w3 sharded 8 ways), L4 row-parallel + AllReduce,
   L5..L14 replicated. Activations kept transposed ([feat, batch]) so no
   on-chip transposes are ever needed.
 - All matmuls run in float32r (fp32 data, FP22 multiply) for full PE speed.
"""

import os
import sys

sys.path.insert(0, "/opt/trn_rl_repo")

import numpy as np

import concourse.bass as bass
import concourse.mybir as mybir
import concourse.tile as tile
from concourse import bacc
from concourse.bass_utils import run_bass_kernel_spmd

F32 = mybir.dt.float32
F32R = mybir.dt.float32r

NCORES = 8
BL = 8              # batch per core
B = 64              # full batch
EPS = 1e-5

LIN_DIMS = [(3200, 2048), (2048, 1024), (1024, 512), (512, 256), (256, 128),
            (128, 64), (64, 32), (32, 16), (16, 8), (8, 4), (4, 2), (2, 1)]

# conv1: [BL,512,40,40] -> [BL,256,14,14]; conv2: -> [BL,128,5,5]
P1 = 196            # 14*14 positions
P2 = 25             # 5*5 positions
NPT = 4             # conv1 psum tiles (2 batch each)
PTW = 2 * P1        # 392 columns per conv1 psum tile

_CACHE = {}


# ----------------------------------------------------------------------------
# device program
# ----------------------------------------------------------------------------

def _build():
    nc = bacc.Bacc("TRN2", target_bir_lowering=False, debug=False,
                   enable_asserts=True, num_devices=NCORES)

    xprep = nc.dram_tensor("xprep", [4, 128, 9, BL * P1], F32, kind="ExternalInput")
    w1p = nc.dram_tensor("w1p", [128, 36, 256], F32, kind="ExternalInput")
    w2p = nc.dram_tensor("w2p", [128, 18, 128], F32, kind="ExternalInput")
    w3p = nc.dram_tensor("w3p", [128, 25, 256], F32, kind="ExternalInput")
    w4p = nc.dram_tensor("w4p", [128, 2, 1024], F32, kind="ExternalInput")
    w5p = nc.dram_tensor("w5p", [128, 8, 512], F32, kind="ExternalInput")
    w6p = nc.dram_tensor("w6p", [128, 4, 256], F32, kind="ExternalInput")
    w7p = nc.dram_tensor("w7p", [128, 2, 128], F32, kind="ExternalInput")
    w8p = nc.dram_tensor("w8p", [128, 1, 64], F32, kind="ExternalInput")
    wtailp = nc.dram_tensor("wtailp", [128, 63], F32, kind="ExternalInput")
    bprep = nc.dram_tensor("bprep", [128, 30], F32, kind="ExternalInput")
    out = nc.dram_tensor("out", [B, 1], F32, kind="ExternalOutput")

    # bprep columns
    BC_BN1G, BC_BN1B, BC_BN2G, BC_BN2B = 0, 2, 4, 5
    BC_B3, BC_B4, BC_B5, BC_B6, BC_B7, BC_B8 = 6, 8, 16, 20, 22, 23
    BC_TAIL = 24  # b9..b14 at cols 24..29

    with tile.TileContext(nc) as tc:
        with tc.tile_pool(name="wp", bufs=1) as wp, \
             tc.tile_pool(name="xp", bufs=3) as xp, \
             tc.tile_pool(name="hp", bufs=1) as hp, \
             tc.tile_pool(name="sp", bufs=1) as sp, \
             tc.tile_pool(name="cps", bufs=4, space="PSUM") as cps, \
             tc.tile_pool(name="c2p", bufs=1, space="PSUM") as c2p, \
             tc.tile_pool(name="mps", bufs=3, space="PSUM") as mps, \
             tc.tile_pool(name="dram", bufs=1, space="DRAM") as dram:

            # ---------------- weight/bias loads -------------------------
            w1sb = wp.tile([128, 36 * 256], F32R)
            nc.sync.dma_start(w1sb[:], w1p.ap().rearrange("p a b -> p (a b)").bitcast(F32R))
            bsb = wp.tile([128, 30], F32)
            nc.sync.dma_start(bsb[:], bprep.ap())

            # ---------------- conv1 -------------------------------------
            h1sb = [hp.tile([128, 4 * PTW], F32R, name=f"h1_{mt}") for mt in range(2)]
            for pt in range(NPT):
                ps = [cps.tile([128, PTW], F32, name="c1ps", tag="c1ps")
                      for _ in range(2)]
                for cb in range(4):
                    xt = xp.tile([128, 9 * PTW], F32R, name="xt", tag="xt")
                    nc.sync.dma_start(
                        xt[:].rearrange("p (k c) -> p k c", k=9),
                        xprep.ap()[cb, :, :, pt * PTW:(pt + 1) * PTW].bitcast(F32R))
                    xtr = xt[:].rearrange("p (k c) -> p k c", k=9)
                    for kij in range(9):
                        rhs = xtr[:, kij]
                        for mt in range(2):
                            lhsT = w1sb[:, (cb * 9 + kij) * 256 + mt * 128:
                                        (cb * 9 + kij) * 256 + (mt + 1) * 128]
                            nc.tensor.matmul(ps[mt][:], lhsT, rhs,
                                             start=(cb == 0 and kij == 0),
                                             stop=(cb == 3 and kij == 8))
                for mt in range(2):
                    nc.vector.tensor_copy(
                        h1sb[mt][:, pt * PTW:(pt + 1) * PTW], ps[mt][:])

            # ---------------- BN1 stats + AllReduce ---------------------
            scratch = sp.tile([128, 1600], F32)
            st_in = sp.tile([128, 4], F32)
            for mt in range(2):
                h = h1sb[mt][:].bitcast(F32)
                nc.vector.reduce_sum(st_in[:, 2 * mt:2 * mt + 1], h,
                                     axis=mybir.AxisListType.X)
                nc.scalar.activation(scratch[:, :4 * PTW], h,
                                     mybir.ActivationFunctionType.Square,
                                     accum_out=st_in[:, 2 * mt + 1:2 * mt + 2])
            bn1_in = dram.tile([128, 4], F32)
            bn1_out = dram.tile([128, 4], F32, addr_space="Shared")
            nc.sync.dma_start(bn1_in[:], st_in[:])
            nc.gpsimd.collective_compute(
                "AllReduce", mybir.AluOpType.add,
                replica_groups=[list(range(NCORES))],
                ins=[bn1_in.opt()], outs=[bn1_out.opt()])
            st1 = sp.tile([128, 4], F32)
            nc.sync.dma_start(st1[:], bn1_out[:])

            # remaining weights (issued after conv1 x-stream)
            w2sb = wp.tile([128, 18 * 128], F32R)
            nc.sync.dma_start(w2sb[:], w2p.ap().rearrange("p a b -> p (a b)").bitcast(F32R))
            w3sb = wp.tile([128, 25 * 256], F32R)
            nc.sync.dma_start(w3sb[:], w3p.ap().rearrange("p a b -> p (a b)").bitcast(F32R))
            w4sb = wp.tile([128, 2 * 1024], F32R)
            nc.sync.dma_start(w4sb[:], w4p.ap().rearrange("p a b -> p (a b)").bitcast(F32R))
            w5sb = wp.tile([128, 8 * 512], F32R)
            nc.sync.dma_start(w5sb[:], w5p.ap().rearrange("p a b -> p (a b)").bitcast(F32R))
            w6sb = wp.tile([128, 4 * 256], F32R)
            nc.sync.dma_start(w6sb[:], w6p.ap().rearrange("p a b -> p (a b)").bitcast(F32R))
            w7sb = wp.tile([128, 2 * 128], F32R)
            nc.sync.dma_start(w7sb[:], w7p.ap().rearrange("p a b -> p (a b)").bitcast(F32R))
            w8sb = wp.tile([128, 64], F32R)
            nc.sync.dma_start(w8sb[:], w8p.ap().rearrange("p a b -> p (a b)").bitcast(F32R))
            wtail = wp.tile([128, 63], F32R)
            nc.sync.dma_start(wtail[:], wtailp.ap().bitcast(F32R))

            # ---------------- BN1 scale/shift + apply -------------------
            def bn_coeffs(pool, stats_sum, stats_sq, count, g_ap, b_ap, name):
                """returns (scale, shift) [p,1] tiles; stats_* are [p,1] APs"""
                p = stats_sum.shape[0]
                t = pool.tile([p, 6], F32, name=f"bn_{name}")
                mean, msq, vpe, sd, r, tn = (t[:, i:i + 1] for i in range(6))
                nc.vector.tensor_scalar(mean, stats_sum, 1.0 / count, None,
                                        op0=mybir.AluOpType.mult)
                nc.vector.tensor_scalar(vpe, stats_sq, 1.0 / count, None,
                                        op0=mybir.AluOpType.mult)
                nc.vector.tensor_tensor(msq, mean, mean, op=mybir.AluOpType.mult)
                nc.vector.tensor_tensor(vpe, vpe, msq, op=mybir.AluOpType.subtract)
                nc.vector.tensor_scalar(vpe, vpe, EPS, None, op0=mybir.AluOpType.add)
                nc.scalar.activation(sd, vpe, mybir.ActivationFunctionType.Sqrt)
                nc.vector.reciprocal(r, sd)
                # one Newton step: r *= 1.5 - 0.5*vpe*r*r
                nc.vector.tensor_tensor(tn, r, r, op=mybir.AluOpType.mult)
                nc.vector.tensor_tensor(tn, tn, vpe, op=mybir.AluOpType.mult)
                nc.vector.tensor_scalar(tn, tn, -0.5, 1.5,
                                        op0=mybir.AluOpType.mult,
                                        op1=mybir.AluOpType.add)
                nc.vector.tensor_tensor(r, r, tn, op=mybir.AluOpType.mult)
                co = pool.tile([p, 2], F32, name=f"bnc_{name}")
                scale, shift = co[:, 0:1], co[:, 1:2]
                nc.vector.tensor_tensor(scale, g_ap, r, op=mybir.AluOpType.mult)
                nc.vector.tensor_tensor(tn, mean, scale, op=mybir.AluOpType.mult)
                nc.vector.tensor_tensor(shift, b_ap, tn, op=mybir.AluOpType.subtract)
                return scale, shift

            for mt in range(2):
                scale, shift = bn_coeffs(
                    sp, st1[:, 2 * mt:2 * mt + 1], st1[:, 2 * mt + 1:2 * mt + 2],
                    B * P1, bsb[:, BC_BN1G + mt:BC_BN1G + mt + 1],
                    bsb[:, BC_BN1B + mt:BC_BN1B + mt + 1], f"bn1_{mt}")
                nc.vector.tensor_scalar(h1sb[mt][:], h1sb[mt][:].bitcast(F32),
                                        scale, shift,
                                        op0=mybir.AluOpType.mult,
                                        op1=mybir.AluOpType.add)

            # ---------------- conv2 (strided APs, boundary split) -------
            # psum layout (i2, j2, n): n innermost keeps fp32r ISA rules happy
            # (innermost count 8 even, outer strides 8/40 even)
            c2ps = c2p.tile([128, P2 * BL], F32)
            c2r = c2ps[:].rearrange("p (i j n) -> p i j n", i=5, j=5, n=BL)
            # full-coverage chunk first so every PSUM element is written once
            kij_order = [(1, 1), (1, 2), (2, 1), (2, 2), (0, 1), (0, 2),
                         (1, 0), (2, 0), (0, 0)]
            nmm = 2 * 9
            cnt = 0
            for cb2 in range(2):
                hr = h1sb[cb2][:].rearrange(
                    "p (n i j) -> p n i j", n=BL, i=14, j=14).transpose([0, 2, 3, 1])
                for (ki, kj) in kij_order:
                    ilo = 1 if ki == 0 else 0
                    jlo = 1 if kj == 0 else 0
                    src = hr[:, 3 * ilo + ki - 1:14:3, 3 * jlo + kj - 1:14:3, :]
                    dst = c2r[:, ilo:, jlo:, :]
                    lhsT = w2sb[:, (cb2 * 9 + ki * 3 + kj) * 128:
                                (cb2 * 9 + ki * 3 + kj + 1) * 128]
                    nc.tensor.matmul(dst, lhsT, src, start=(cnt == 0),
                                     stop=(cnt == nmm - 1), skip_group_check=True)
                    cnt += 1
            c2sb = sp.tile([128, BL * P2], F32)
            nc.vector.tensor_copy(c2sb[:], c2ps[:])

            # ---------------- AllGather conv2 raw -----------------------
            ag_in = dram.tile([128, BL * P2], F32)
            ag_out = dram.tile([NCORES, 128, BL * P2], F32, addr_space="Shared")
            nc.sync.dma_start(ag_in[:], c2sb[:])
            nc.gpsimd.collective_compute(
                "AllGather", mybir.AluOpType.bypass,
                replica_groups=[list(range(NCORES))],
                ins=[ag_in.opt()], outs=[ag_out.opt()])
            g2 = sp.tile([128, B * P2], F32R)
            nc.sync.dma_start(
                g2[:].rearrange("p (r t) -> p r t", r=NCORES),
                bass.AP(ag_out.tensor, 0,
                        [[BL * P2, 128], [128 * BL * P2, NCORES], [1, BL * P2]]
                        ).bitcast(F32R))

            # ---------------- BN2 (redundant, full batch) ---------------
            st2 = sp.tile([128, 2], F32)
            g2f = g2[:].bitcast(F32)
            nc.vector.reduce_sum(st2[:, 0:1], g2f, axis=mybir.AxisListType.X)
            nc.scalar.activation(scratch[:, :B * P2], g2f,
                                 mybir.ActivationFunctionType.Square,
                                 accum_out=st2[:, 1:2])
            scale2, shift2 = bn_coeffs(
                sp, st2[:, 0:1], st2[:, 1:2], B * P2,
                bsb[:, BC_BN2G:BC_BN2G + 1], bsb[:, BC_BN2B:BC_BN2B + 1], "bn2")
            nc.vector.tensor_scalar(g2[:], g2f, scale2, shift2,
                                    op0=mybir.AluOpType.mult,
                                    op1=mybir.AluOpType.add)

            # ---------------- MLP (transposed activations) --------------
            # g2 free layout is (r, ij, n8); (r, n8) = global batch order
            g2n = g2[:].rearrange("p (r i n) -> p r i n", r=NCORES, i=P2)

            # L3: y3T [2][128, 64], this core's 256-feature slice
            y3T = sp.tile([128, 2 * B], F32R)
            for mt in range(2):
                ps = mps.tile([128, B], F32, name="mlpps", tag="mlpps")
                for ij in range(25):
                    lhsT = w3sb[:, ij * 256 + mt * 128: ij * 256 + (mt + 1) * 128]
                    nc.tensor.matmul(ps[:], lhsT, g2n[:, :, ij, :],
                                     start=(ij == 0), stop=(ij == 24))
                nc.vector.tensor_scalar(y3T[:, mt * B:(mt + 1) * B], ps[:],
                                        bsb[:, BC_B3 + mt:BC_B3 + mt + 1], None,
                                        op0=mybir.AluOpType.add)

            # L4 partial: y4pT [128, 8*64], feature f = mt*128 + p
            y4p = sp.tile([128, 8 * B], F32)
            for mt in range(8):
                ps = mps.tile([128, B], F32, name="mlpps", tag="mlpps")
                for kc in range(2):
                    lhsT = w4sb[:, kc * 1024 + mt * 128: kc * 1024 + (mt + 1) * 128]
                    nc.tensor.matmul(ps[:], lhsT, y3T[:, kc * B:(kc + 1) * B],
                                     start=(kc == 0), stop=(kc == 1))
                nc.vector.tensor_copy(y4p[:, mt * B:(mt + 1) * B], ps[:])

            ar_in = dram.tile([128, 8 * B], F32)
            ar_out = dram.tile([128, 8 * B], F32, addr_space="Shared")
            nc.sync.dma_start(ar_in[:], y4p[:])
            nc.gpsimd.collective_compute(
                "AllReduce", mybir.AluOpType.add,
                replica_groups=[list(range(NCORES))],
                ins=[ar_in.opt()], outs=[ar_out.opt()])
            y4s = sp.tile([128, 8 * B], F32)
            nc.sync.dma_start(y4s[:], ar_out[:])
            y4T = sp.tile([128, 8 * B], F32R)
            for mt in range(8):
                nc.vector.tensor_scalar(y4T[:, mt * B:(mt + 1) * B],
                                        y4s[:, mt * B:(mt + 1) * B],
                                        bsb[:, BC_B4 + mt:BC_B4 + mt + 1], None,
                                        op0=mybir.AluOpType.add)

            # L5..L7: uniform pattern
            def mid_layer(yprev, wsb, nmt, nkc, bcol, name):
                ynext = sp.tile([128, nmt * B], F32R, name=name)
                for mt in range(nmt):
                    ps = mps.tile([128, B], F32, name="mlpps", tag="mlpps")
                    for kc in range(nkc):
                        lhsT = wsb[:, kc * (nmt * 128) + mt * 128:
                                   kc * (nmt * 128) + (mt + 1) * 128]
                        nc.tensor.matmul(ps[:], lhsT, yprev[:, kc * B:(kc + 1) * B],
                                         start=(kc == 0), stop=(kc == nkc - 1))
                    nc.vector.tensor_scalar(ynext[:, mt * B:(mt + 1) * B], ps[:],
                                            bsb[:, bcol + mt:bcol + mt + 1], None,
                                            op0=mybir.AluOpType.add)
                return ynext

            y5T = mid_layer(y4T, w5sb, 4, 8, BC_B5, "y5T")
            y6T = mid_layer(y5T, w6sb, 2, 4, BC_B6, "y6T")
            y7T = mid_layer(y6T, w7sb, 1, 2, BC_B7, "y7T")

            # L8: 128 -> 64
            ps8 = mps.tile([64, B], F32, name="ps8", tag="mlpps")
            nc.tensor.matmul(ps8[:], w8sb[:, 0:64], y7T[:, 0:B],
                             start=True, stop=True)
            y8T = sp.tile([64, B], F32R)
            nc.vector.tensor_scalar(y8T[:], ps8[:], bsb[0:64, BC_B8:BC_B8 + 1],
                                    None, op0=mybir.AluOpType.add)

            # tail: 64->32->16->8->4->2->1 from packed wtail
            dims = [64, 32, 16, 8, 4, 2, 1]
            col = 0
            yprev = y8T
            for li in range(6):
                din, dout = dims[li], dims[li + 1]
                ps = mps.tile([dout, B], F32, name=f"pst{li}", tag="mlpps")
                nc.tensor.matmul(ps[:], wtail[0:din, col:col + dout], yprev[:],
                                 start=True, stop=True)
                yn = sp.tile([dout, B], F32R, name=f"yt{li}")
                nc.vector.tensor_scalar(
                    yn[:], ps[:], bsb[0:dout, BC_TAIL + li:BC_TAIL + li + 1],
                    None, op0=mybir.AluOpType.add)
                col += dout
                yprev = yn

            # sigmoid + output
            osb = sp.tile([1, B], F32)
            nc.scalar.activation(osb[:], yprev[:].bitcast(F32),
                                 mybir.ActivationFunctionType.Sigmoid)
            nc.sync.dma_start(bass.AP(out, 0, [[1, 1], [1, B]]), osb[:])

    nc.compile()
    return nc


# ----------------------------------------------------------------------------
# host-side input prep
# ----------------------------------------------------------------------------

def _prep_inputs(inputs):
    f = np.float32
    x = np.asarray(inputs["x"], dtype=f)

    # conv1 patches, per core: [4cb, 128c, 3ki, 3kj, 8n, 14i, 14j]
    xpad = np.zeros((B, 512, 42, 42), dtype=f)
    xpad[:, :, 1:41, 1:41] = x
    xv = xpad.reshape(B, 4, 128, 14, 3, 14, 3).transpose(1, 2, 4, 6, 0, 3, 5)

    w1 = np.asarray(inputs["conv1_w"], dtype=f)          # [256, 512, 3, 3]
    w1p = np.ascontiguousarray(
        w1.reshape(256, 4, 128, 9).transpose(2, 1, 3, 0)).reshape(128, 36, 256)
    w2 = np.asarray(inputs["conv2_w"], dtype=f)          # [128, 256, 3, 3]
    w2p = np.ascontiguousarray(
        w2.reshape(128, 2, 128, 9).transpose(2, 1, 3, 0)).reshape(128, 18, 128)

    ws = [np.asarray(inputs[f"w{i+3}"], dtype=f) for i in range(12)]
    bs = [np.asarray(inputs[f"b{i+3}"], dtype=f) for i in range(12)]
    w3, w4, w5, w6, w7, w8 = ws[0], ws[1], ws[2], ws[3], ws[4], ws[5]

    w4p = np.ascontiguousarray(w4.reshape(1024, 8, 2, 128).transpose(1, 3, 2, 0))
    # per-core slice r: w4[:, r*256 + kc*128 + c] -> [8r][128c, 2kc, 1024m]
    w5p = np.ascontiguousarray(w5.reshape(512, 8, 128).transpose(2, 1, 0))
    w6p = np.ascontiguousarray(w6.reshape(256, 4, 128).transpose(2, 1, 0))
    w7p = np.ascontiguousarray(w7.reshape(128, 2, 128).transpose(2, 1, 0))
    w8p = np.ascontiguousarray(w8.reshape(64, 1, 128).transpose(2, 1, 0))

    wtail = np.zeros((128, 63), dtype=f)
    col = 0
    for wl in ws[6:]:                       # w9..w14: [dout, din]
        dout, din = wl.shape
        wtail[0:din, col:col + dout] = wl.T
        col += dout

    bn1_g = np.asarray(inputs["bn1_g"], dtype=f)
    bn1_b = np.asarray(inputs["bn1_b"], dtype=f)
    bn2_g = np.asarray(inputs["bn2_g"], dtype=f)
    bn2_b = np.asarray(inputs["bn2_b"], dtype=f)

    def bpack(core):
        bp = np.zeros((128, 30), dtype=f)
        bp[:, 0:2] = bn1_g.reshape(2, 128).T
        bp[:, 2:4] = bn1_b.reshape(2, 128).T
        bp[:, 4] = bn2_g
        bp[:, 5] = bn2_b
        bp[:, 6:8] = bs[0][core * 256:(core + 1) * 256].reshape(2, 128).T
        bp[:, 8:16] = bs[1].reshape(8, 128).T
        bp[:, 16:20] = bs[2].reshape(4, 128).T
        bp[:, 20:22] = bs[3].reshape(2, 128).T
        bp[:, 22] = bs[4]
        bp[0:64, 23] = bs[5]
        for li in range(6):
            d = bs[6 + li].shape[0]
            bp[0:d, 24 + li] = bs[6 + li]
        return bp

    in_maps = []
    for r in range(NCORES):
        xr = np.ascontiguousarray(xv[:, :, :, :, r * BL:(r + 1) * BL]
                                  ).reshape(4, 128, 9, BL * P1)
        # w3p[c, ij, m] = w3[r*256 + m, c*25 + ij]
        w3r = np.ascontiguousarray(
            w3[r * 256:(r + 1) * 256].reshape(256, 128, 25).transpose(1, 2, 0))
        in_maps.append({
            "xprep": xr,
            "w1p": w1p, "w2p": w2p,
            "w3p": w3r,
            "w4p": np.ascontiguousarray(w4p[r]),
            "w5p": w5p, "w6p": w6p, "w7p": w7p, "w8p": w8p,
            "wtailp": wtail, "bprep": bpack(r),
        })
    return in_maps


def kernel(**inputs):
    if "nc" not in _CACHE:
        _CACHE["nc"] = _build()
    nc = _CACHE["nc"]
    in_maps = _prep_inputs(inputs)
    trace = bool(int(os.environ.get("KERNEL_TRACE", "0")))
    if trace:
        import ntff_shim
        ntff_shim.install()
    res = run_bass_kernel_spmd(nc, in_maps, core_ids=list(range(NCORES)),
                               trace=trace)
    _CACHE["last_result"] = res
    return res.results[0]["out"]


# revision 7
# speedup vs baseline: 1.5708x; 1.5708x over previous
"""Trainium2 Bass kernel for nn_DomainDiscriminator.

Network: conv(512->256,k3,s3,p1) -> BN -> conv(256->128,k3,s3,p1) -> BN
         -> reshape -> 12-layer MLP (3200->2048->...->1) -> sigmoid.
Input x: [64, 512, 40, 40] f32.  Output: [64, 1] f32.

Strategy (8 NeuronCores):
 - Data-parallel batch shard (8 per core) for the convs.
 - stride==kernel==3 convs are non-overlapping patch matmuls. Conv1 patches are
   built host-side (space-to-depth, free); conv2 patches are read straight out
   of SBUF with strided access patterns (boundary-split matmuls, no im2col).
 - Training-mode BN: conv bias is absorbed exactly by BN; BN1 stats via a 2KB
   AllReduce; BN2 stats computed redundantly after an AllGather of the conv2
   raw output (which the MLP needs anyway).
 - MLP: L3 column-parallel (w3 sharded 8 ways), L4 row-parallel + AllReduce,
   L5..L14 replicated. Activations kept transposed ([feat, batch]) so no
   on-chip transposes are ever needed.
 - All matmuls run in float32r (fp32 data, FP22 multiply) for full PE speed.
"""

import os
import sys

sys.path.insert(0, "/opt/trn_rl_repo")

import numpy as np

import concourse.bass as bass
import concourse.mybir as mybir
import concourse.tile as tile
from concourse import bacc
from concourse.bass_utils import run_bass_kernel_spmd

F32 = mybir.dt.float32
F32R = mybir.dt.float32r
BF16 = mybir.dt.bfloat16

NCORES = 8
BL = 8              # batch per core
B = 64              # full batch
EPS = 1e-5

LIN_DIMS = [(3200, 2048), (2048, 1024), (1024, 512), (512, 256), (256, 128),
            (128, 64), (64, 32), (32, 16), (16, 8), (8, 4), (4, 2), (2, 1)]

# conv1: [BL,512,40,40] -> [BL,256,14,14]; conv2: -> [BL,128,5,5]
P1 = 196            # 14*14 positions
P2 = 25             # 5*5 positions
NPT = 4             # conv1 psum tiles (2 batch each)
PTW = 2 * P1        # 392 columns per conv1 psum tile

_CACHE = {}


# ----------------------------------------------------------------------------
# device program
# ----------------------------------------------------------------------------

def _build():
    nc = bacc.Bacc("TRN2", target_bir_lowering=False, debug=False,
                   enable_asserts=True, num_devices=NCORES)

    xprep = nc.dram_tensor("xprep", [4, 128, 9, BL * P1], BF16, kind="ExternalInput")
    w1p = nc.dram_tensor("w1p", [128, 36, 256], BF16, kind="ExternalInput")
    w2p = nc.dram_tensor("w2p", [128, 18, 128], BF16, kind="ExternalInput")
    w3p = nc.dram_tensor("w3p", [128, 25, 256], F32, kind="ExternalInput")
    w4p = nc.dram_tensor("w4p", [128, 2, 1024], F32, kind="ExternalInput")
    w5p = nc.dram_tensor("w5p", [128, 8, 512], F32, kind="ExternalInput")
    w6p = nc.dram_tensor("w6p", [128, 4, 256], F32, kind="ExternalInput")
    w7p = nc.dram_tensor("w7p", [128, 2, 128], F32, kind="ExternalInput")
    w8p = nc.dram_tensor("w8p", [128, 1, 64], F32, kind="ExternalInput")
    wtailp = nc.dram_tensor("wtailp", [128, 63], F32, kind="ExternalInput")
    bprep = nc.dram_tensor("bprep", [128, 30], F32, kind="ExternalInput")
    out = nc.dram_tensor("out", [B, 1], F32, kind="ExternalOutput")
    debug = bool(int(os.environ.get("KERNEL_DEBUG", "0")))
    if debug:
        dbg_h1 = nc.dram_tensor("dbg_h1", [2, 128, 1568], F32, kind="ExternalOutput")
        dbg_g2 = nc.dram_tensor("dbg_g2", [128, 1600], F32, kind="ExternalOutput")
        dbg_st1 = nc.dram_tensor("dbg_st1", [128, 4], F32, kind="ExternalOutput")
        dbg_y3 = nc.dram_tensor("dbg_y3", [128, 2 * B], F32, kind="ExternalOutput")
        dbg_y4 = nc.dram_tensor("dbg_y4", [128, 8 * B], F32, kind="ExternalOutput")

    # bprep columns
    BC_BN1G, BC_BN1B, BC_BN2G, BC_BN2B = 0, 2, 4, 5
    BC_B3, BC_B4, BC_B5, BC_B6, BC_B7, BC_B8 = 6, 8, 16, 20, 22, 23
    BC_TAIL = 24  # b9..b14 at cols 24..29

    with tile.TileContext(nc) as tc:
        with tc.tile_pool(name="wp", bufs=1) as wp, \
             tc.tile_pool(name="xp", bufs=3) as xp, \
             tc.tile_pool(name="hp", bufs=1) as hp, \
             tc.tile_pool(name="sp", bufs=1) as sp, \
             tc.tile_pool(name="cps", bufs=4, space="PSUM") as cps, \
             tc.tile_pool(name="c2p", bufs=1, space="PSUM") as c2p, \
             tc.tile_pool(name="mps", bufs=3, space="PSUM") as mps, \
             tc.tile_pool(name="dram", bufs=1, space="DRAM") as dram:

            # ---------------- weight/bias loads -------------------------
            w1sb = wp.tile([128, 36 * 256], BF16)
            nc.sync.dma_start(w1sb[:], w1p.ap().rearrange("p a b -> p (a b)"))
            bsb = wp.tile([128, 30], F32)
            nc.sync.dma_start(bsb[:], bprep.ap())

            # ---------------- conv1 -------------------------------------
            h1sb = [hp.tile([128, 4 * PTW], BF16, name=f"h1_{mt}") for mt in range(2)]
            for pt in range(NPT):
                ps = [cps.tile([128, PTW], F32, name="c1ps", tag="c1ps")
                      for _ in range(2)]
                for cb in range(4):
                    xt = xp.tile([128, 9 * PTW], BF16, name="xt", tag="xt")
                    nc.sync.dma_start(
                        xt[:].rearrange("p (k c) -> p k c", k=9),
                        xprep.ap()[cb, :, :, pt * PTW:(pt + 1) * PTW])
                    xtr = xt[:].rearrange("p (k c) -> p k c", k=9)
                    for kij in range(9):
                        rhs = xtr[:, kij]
                        for mt in range(2):
                            lhsT = w1sb[:, (cb * 9 + kij) * 256 + mt * 128:
                                        (cb * 9 + kij) * 256 + (mt + 1) * 128]
                            nc.tensor.matmul(ps[mt][:], lhsT, rhs,
                                             start=(cb == 0 and kij == 0),
                                             stop=(cb == 3 and kij == 8))
                for mt in range(2):
                    nc.vector.tensor_copy(
                        h1sb[mt][:, pt * PTW:(pt + 1) * PTW], ps[mt][:])

            # ---------------- BN1 stats + AllReduce ---------------------
            scratch = sp.tile([128, 1600], F32)
            st_in = sp.tile([128, 4], F32)
            for mt in range(2):
                h = h1sb[mt][:]
                nc.vector.reduce_sum(st_in[:, 2 * mt:2 * mt + 1], h,
                                     axis=mybir.AxisListType.X)
                nc.scalar.activation(scratch[:, :4 * PTW], h,
                                     mybir.ActivationFunctionType.Square,
                                     accum_out=st_in[:, 2 * mt + 1:2 * mt + 2])
            bn1_in = dram.tile([128, 4], F32)
            bn1_out = dram.tile([128, 4], F32, addr_space="Shared")
            nc.sync.dma_start(bn1_in[:], st_in[:])
            nc.gpsimd.collective_compute(
                "AllReduce", mybir.AluOpType.add,
                replica_groups=[list(range(NCORES))],
                ins=[bn1_in.opt()], outs=[bn1_out.opt()])
            st1 = sp.tile([128, 4], F32)
            nc.sync.dma_start(st1[:], bn1_out[:])

            # remaining weights (issued after conv1 x-stream)
            w2sb = wp.tile([128, 18 * 128], BF16)
            nc.sync.dma_start(w2sb[:], w2p.ap().rearrange("p a b -> p (a b)"))
            w3sb = wp.tile([128, 25 * 256], F32R)
            nc.sync.dma_start(w3sb[:], w3p.ap().rearrange("p a b -> p (a b)").bitcast(F32R))
            w4sb = wp.tile([128, 2 * 1024], F32R)
            nc.sync.dma_start(w4sb[:], w4p.ap().rearrange("p a b -> p (a b)").bitcast(F32R))
            w5sb = wp.tile([128, 8 * 512], F32R)
            nc.sync.dma_start(w5sb[:], w5p.ap().rearrange("p a b -> p (a b)").bitcast(F32R))
            w6sb = wp.tile([128, 4 * 256], F32R)
            nc.sync.dma_start(w6sb[:], w6p.ap().rearrange("p a b -> p (a b)").bitcast(F32R))
            w7sb = wp.tile([128, 2 * 128], F32R)
            nc.sync.dma_start(w7sb[:], w7p.ap().rearrange("p a b -> p (a b)").bitcast(F32R))
            w8sb = wp.tile([128, 64], F32R)
            nc.sync.dma_start(w8sb[:], w8p.ap().rearrange("p a b -> p (a b)").bitcast(F32R))
            wtail = wp.tile([128, 63], F32R)
            nc.sync.dma_start(wtail[:], wtailp.ap().bitcast(F32R))

            # ---------------- BN1 scale/shift + apply -------------------
            def bn_coeffs(pool, stats_sum, stats_sq, count, g_ap, b_ap, name):
                """returns (scale, shift) [p,1] tiles; stats_* are [p,1] APs"""
                p = stats_sum.shape[0]
                t = pool.tile([p, 6], F32, name=f"bn_{name}")
                mean, msq, vpe, sd, r, tn = (t[:, i:i + 1] for i in range(6))
                nc.vector.tensor_scalar(mean, stats_sum, 1.0 / count, None,
                                        op0=mybir.AluOpType.mult)
                nc.vector.tensor_scalar(vpe, stats_sq, 1.0 / count, None,
                                        op0=mybir.AluOpType.mult)
                nc.vector.tensor_tensor(msq, mean, mean, op=mybir.AluOpType.mult)
                nc.vector.tensor_tensor(vpe, vpe, msq, op=mybir.AluOpType.subtract)
                nc.vector.tensor_scalar(vpe, vpe, EPS, None, op0=mybir.AluOpType.add)
                nc.scalar.activation(sd, vpe, mybir.ActivationFunctionType.Sqrt)
                nc.vector.reciprocal(r, sd)
                # one Newton step: r *= 1.5 - 0.5*vpe*r*r
                nc.vector.tensor_tensor(tn, r, r, op=mybir.AluOpType.mult)
                nc.vector.tensor_tensor(tn, tn, vpe, op=mybir.AluOpType.mult)
                nc.vector.tensor_scalar(tn, tn, -0.5, 1.5,
                                        op0=mybir.AluOpType.mult,
                                        op1=mybir.AluOpType.add)
                nc.vector.tensor_tensor(r, r, tn, op=mybir.AluOpType.mult)
                co = pool.tile([p, 2], F32, name=f"bnc_{name}")
                scale, shift = co[:, 0:1], co[:, 1:2]
                nc.vector.tensor_tensor(scale, g_ap, r, op=mybir.AluOpType.mult)
                nc.vector.tensor_tensor(tn, mean, scale, op=mybir.AluOpType.mult)
                nc.vector.tensor_tensor(shift, b_ap, tn, op=mybir.AluOpType.subtract)
                return scale, shift

            for mt in range(2):
                scale, shift = bn_coeffs(
                    sp, st1[:, 2 * mt:2 * mt + 1], st1[:, 2 * mt + 1:2 * mt + 2],
                    B * P1, bsb[:, BC_BN1G + mt:BC_BN1G + mt + 1],
                    bsb[:, BC_BN1B + mt:BC_BN1B + mt + 1], f"bn1_{mt}")
                nc.vector.tensor_scalar(h1sb[mt][:], h1sb[mt][:],
                                        scale, shift,
                                        op0=mybir.AluOpType.mult,
                                        op1=mybir.AluOpType.add)

            if debug:
                for mt in range(2):
                    dh = sp.tile([128, 1568], F32, name=f"dh{mt}")
                    nc.vector.tensor_copy(dh[:], h1sb[mt][:])
                    nc.sync.dma_start(dbg_h1.ap()[mt], dh[:])
                nc.sync.dma_start(dbg_st1.ap(), st1[:])

            # ---------------- conv2 (strided APs, boundary split) -------
            # psum layout (i2, j2, n): n innermost keeps fp32r ISA rules happy
            # (innermost count 8 even, outer strides 8/40 even)
            c2ps = c2p.tile([128, P2 * BL], F32)
            c2r = c2ps[:].rearrange("p (i j n) -> p i j n", i=5, j=5, n=BL)
            # full-coverage chunk first so every PSUM element is written once
            kij_order = [(1, 1), (1, 2), (2, 1), (2, 2), (0, 1), (0, 2),
                         (1, 0), (2, 0), (0, 0)]
            nmm = 2 * 9
            cnt = 0
            for cb2 in range(2):
                hr = h1sb[cb2][:].rearrange(
                    "p (n i j) -> p n i j", n=BL, i=14, j=14).transpose([0, 2, 3, 1])
                for (ki, kj) in kij_order:
                    ilo = 1 if ki == 0 else 0
                    jlo = 1 if kj == 0 else 0
                    src = hr[:, 3 * ilo + ki - 1:14:3, 3 * jlo + kj - 1:14:3, :]
                    dst = c2r[:, ilo:, jlo:, :]
                    lhsT = w2sb[:, (cb2 * 9 + ki * 3 + kj) * 128:
                                (cb2 * 9 + ki * 3 + kj + 1) * 128]
                    nc.tensor.matmul(dst, lhsT, src, start=(cnt == 0),
                                     stop=(cnt == nmm - 1), skip_group_check=True)
                    cnt += 1
            c2sb = sp.tile([128, BL * P2], BF16)
            nc.vector.tensor_copy(c2sb[:], c2ps[:])

            # ---------------- AllGather conv2 raw -----------------------
            ag_in = dram.tile([128, BL * P2], BF16)
            ag_out = dram.tile([NCORES, 128, BL * P2], BF16, addr_space="Shared")
            nc.sync.dma_start(ag_in[:], c2sb[:])
            nc.gpsimd.collective_compute(
                "AllGather", mybir.AluOpType.bypass,
                replica_groups=[list(range(NCORES))],
                ins=[ag_in.opt()], outs=[ag_out.opt()])
            g2 = sp.tile([128, B * P2], BF16)
            nc.sync.dma_start(
                g2[:].rearrange("p (r t) -> p r t", r=NCORES),
                bass.AP(ag_out.tensor, 0,
                        [[BL * P2, 128], [128 * BL * P2, NCORES], [1, BL * P2]]))

            # ---------------- BN2 (redundant, full batch) ---------------
            st2 = sp.tile([128, 2], F32)
            nc.vector.reduce_sum(st2[:, 0:1], g2[:], axis=mybir.AxisListType.X)
            nc.scalar.activation(scratch[:, :B * P2], g2[:],
                                 mybir.ActivationFunctionType.Square,
                                 accum_out=st2[:, 1:2])
            scale2, shift2 = bn_coeffs(
                sp, st2[:, 0:1], st2[:, 1:2], B * P2,
                bsb[:, BC_BN2G:BC_BN2G + 1], bsb[:, BC_BN2B:BC_BN2B + 1], "bn2")
            g2a = sp.tile([128, B * P2], F32R)
            nc.vector.tensor_scalar(g2a[:], g2[:], scale2, shift2,
                                    op0=mybir.AluOpType.mult,
                                    op1=mybir.AluOpType.add)

            if debug:
                dg = sp.tile([128, 1600], F32, name="dg")
                nc.vector.tensor_copy(dg[:], g2a[:].bitcast(F32))
                nc.sync.dma_start(dbg_g2.ap(), dg[:])

            # ---------------- MLP (transposed activations) --------------
            # g2 free layout is (r, ij, n8); (r, n8) = global batch order
            g2n = g2a[:].rearrange("p (r i n) -> p r i n", r=NCORES, i=P2)

            # L3: y3T [2][128, 64], this core's 256-feature slice
            y3T = sp.tile([128, 2 * B], F32R)
            for mt in range(2):
                ps = mps.tile([128, B], F32, name="mlpps", tag="mlpps")
                for ij in range(25):
                    lhsT = w3sb[:, ij * 256 + mt * 128: ij * 256 + (mt + 1) * 128]
                    nc.tensor.matmul(ps[:], lhsT, g2n[:, :, ij, :],
                                     start=(ij == 0), stop=(ij == 24))
                nc.vector.tensor_scalar(y3T[:, mt * B:(mt + 1) * B], ps[:],
                                        bsb[:, BC_B3 + mt:BC_B3 + mt + 1], None,
                                        op0=mybir.AluOpType.add)

            # L4 partial: y4pT [128, 8*64], feature f = mt*128 + p
            y4p = sp.tile([128, 8 * B], F32)
            for mt in range(8):
                ps = mps.tile([128, B], F32, name="mlpps", tag="mlpps")
                for kc in range(2):
                    lhsT = w4sb[:, kc * 1024 + mt * 128: kc * 1024 + (mt + 1) * 128]
                    nc.tensor.matmul(ps[:], lhsT, y3T[:, kc * B:(kc + 1) * B],
                                     start=(kc == 0), stop=(kc == 1))
                nc.vector.tensor_copy(y4p[:, mt * B:(mt + 1) * B], ps[:])

            if debug:
                dy3 = sp.tile([128, 2 * B], F32, name="dy3")
                nc.vector.tensor_copy(dy3[:], y3T[:].bitcast(F32))
                nc.sync.dma_start(dbg_y3.ap(), dy3[:])
            ar_in = dram.tile([128, 8 * B], F32)
            ar_out = dram.tile([128, 8 * B], F32, addr_space="Shared")
            nc.sync.dma_start(ar_in[:], y4p[:])
            nc.gpsimd.collective_compute(
                "AllReduce", mybir.AluOpType.add,
                replica_groups=[list(range(NCORES))],
                ins=[ar_in.opt()], outs=[ar_out.opt()])
            y4s = sp.tile([128, 8 * B], F32)
            nc.sync.dma_start(y4s[:], ar_out[:])
            if debug:
                nc.sync.dma_start(dbg_y4.ap(), y4s[:])
            y4T = sp.tile([128, 8 * B], F32R)
            for mt in range(8):
                nc.vector.tensor_scalar(y4T[:, mt * B:(mt + 1) * B],
                                        y4s[:, mt * B:(mt + 1) * B],
                                        bsb[:, BC_B4 + mt:BC_B4 + mt + 1], None,
                                        op0=mybir.AluOpType.add)

            # L5..L7: uniform pattern
            def mid_layer(yprev, wsb, nmt, nkc, bcol, name):
                ynext = sp.tile([128, nmt * B], F32R, name=name)
                for mt in range(nmt):
                    ps = mps.tile([128, B], F32, name="mlpps", tag="mlpps")
                    for kc in range(nkc):
                        lhsT = wsb[:, kc * (nmt * 128) + mt * 128:
                                   kc * (nmt * 128) + (mt + 1) * 128]
                        nc.tensor.matmul(ps[:], lhsT, yprev[:, kc * B:(kc + 1) * B],
                                         start=(kc == 0), stop=(kc == nkc - 1))
                    nc.vector.tensor_scalar(ynext[:, mt * B:(mt + 1) * B], ps[:],
                                            bsb[:, bcol + mt:bcol + mt + 1], None,
                                            op0=mybir.AluOpType.add)
                return ynext

            y5T = mid_layer(y4T, w5sb, 4, 8, BC_B5, "y5T")
            y6T = mid_layer(y5T, w6sb, 2, 4, BC_B6, "y6T")
            y7T = mid_layer(y6T, w7sb, 1, 2, BC_B7, "y7T")

            # L8: 128 -> 64
            ps8 = mps.tile([64, B], F32, name="ps8", tag="mlpps")
            nc.tensor.matmul(ps8[:], w8sb[:, 0:64], y7T[:, 0:B],
                             start=True, stop=True)
            y8T = sp.tile([64, B], F32R)
            nc.vector.tensor_scalar(y8T[:], ps8[:], bsb[0:64, BC_B8:BC_B8 + 1],
                                    None, op0=mybir.AluOpType.add)

            # tail: 64->32->16->8->4->2->1 from packed wtail
            dims = [64, 32, 16, 8, 4, 2, 1]
            col = 0
            yprev = y8T
            for li in range(6):
                din, dout = dims[li], dims[li + 1]
                ps = mps.tile([dout, B], F32, name=f"pst{li}", tag="mlpps")
                nc.tensor.matmul(ps[:], wtail[0:din, col:col + dout], yprev[:],
                                 start=True, stop=True)
                yn = sp.tile([dout, B], F32R, name=f"yt{li}")
                nc.vector.tensor_scalar(
                    yn[:], ps[:], bsb[0:dout, BC_TAIL + li:BC_TAIL + li + 1],
                    None, op0=mybir.AluOpType.add)
                col += dout
                yprev = yn

            # sigmoid + output
            osb = sp.tile([1, B], F32)
            nc.scalar.activation(osb[:], yprev[:].bitcast(F32),
                                 mybir.ActivationFunctionType.Sigmoid)
            nc.sync.dma_start(bass.AP(out, 0, [[1, 1], [1, B]]), osb[:])

    nc.compile()
    return nc


# ----------------------------------------------------------------------------
# host-side input prep
# ----------------------------------------------------------------------------

def _prep_inputs(inputs):
    import ml_dtypes
    f = np.float32
    bf = ml_dtypes.bfloat16
    x = np.asarray(inputs["x"], dtype=f)

    # conv1 patches, per core: [4cb, 128c, 3ki, 3kj, 8n, 14i, 14j]
    xpad = np.zeros((B, 512, 42, 42), dtype=bf)
    xpad[:, :, 1:41, 1:41] = x.astype(bf)
    xv = xpad.reshape(B, 4, 128, 14, 3, 14, 3).transpose(1, 2, 4, 6, 0, 3, 5)

    w1 = np.asarray(inputs["conv1_w"], dtype=f)          # [256, 512, 3, 3]
    w1p = np.ascontiguousarray(
        w1.reshape(256, 4, 128, 9).transpose(2, 1, 3, 0)).reshape(128, 36, 256).astype(bf)
    w2 = np.asarray(inputs["conv2_w"], dtype=f)          # [128, 256, 3, 3]
    w2p = np.ascontiguousarray(
        w2.reshape(128, 2, 128, 9).transpose(2, 1, 3, 0)).reshape(128, 18, 128).astype(bf)

    ws = [np.asarray(inputs[f"w{i+3}"], dtype=f) for i in range(12)]
    bs = [np.asarray(inputs[f"b{i+3}"], dtype=f) for i in range(12)]
    w3, w4, w5, w6, w7, w8 = ws[0], ws[1], ws[2], ws[3], ws[4], ws[5]

    w4p = np.ascontiguousarray(w4.reshape(1024, 8, 2, 128).transpose(1, 3, 2, 0))
    # per-core slice r: w4[:, r*256 + kc*128 + c] -> [8r][128c, 2kc, 1024m]
    w5p = np.ascontiguousarray(w5.reshape(512, 8, 128).transpose(2, 1, 0))
    w6p = np.ascontiguousarray(w6.reshape(256, 4, 128).transpose(2, 1, 0))
    w7p = np.ascontiguousarray(w7.reshape(128, 2, 128).transpose(2, 1, 0))
    w8p = np.ascontiguousarray(w8.reshape(64, 1, 128).transpose(2, 1, 0))

    wtail = np.zeros((128, 63), dtype=f)
    col = 0
    for wl in ws[6:]:                       # w9..w14: [dout, din]
        dout, din = wl.shape
        wtail[0:din, col:col + dout] = wl.T
        col += dout

    bn1_g = np.asarray(inputs["bn1_g"], dtype=f)
    bn1_b = np.asarray(inputs["bn1_b"], dtype=f)
    bn2_g = np.asarray(inputs["bn2_g"], dtype=f)
    bn2_b = np.asarray(inputs["bn2_b"], dtype=f)

    def bpack(core):
        bp = np.zeros((128, 30), dtype=f)
        bp[:, 0:2] = bn1_g.reshape(2, 128).T
        bp[:, 2:4] = bn1_b.reshape(2, 128).T
        bp[:, 4] = bn2_g
        bp[:, 5] = bn2_b
        bp[:, 6:8] = bs[0][core * 256:(core + 1) * 256].reshape(2, 128).T
        bp[:, 8:16] = bs[1].reshape(8, 128).T
        bp[:, 16:20] = bs[2].reshape(4, 128).T
        bp[:, 20:22] = bs[3].reshape(2, 128).T
        bp[:, 22] = bs[4]
        bp[0:64, 23] = bs[5]
        for li in range(6):
            d = bs[6 + li].shape[0]
            bp[0:d, 24 + li] = bs[6 + li]
        return bp

    in_maps = []
    for r in range(NCORES):
        xr = np.ascontiguousarray(xv[:, :, :, :, r * BL:(r + 1) * BL]
                                  ).reshape(4, 128, 9, BL * P1)
        # w3p[c, ij, m] = w3[r*256 + m, c*25 + ij]
        w3r = np.ascontiguousarray(
            w3[r * 256:(r + 1) * 256].reshape(256, 128, 25).transpose(1, 2, 0))
        in_maps.append({
            "xprep": xr,
            "w1p": w1p, "w2p": w2p,
            "w3p": w3r,
            "w4p": np.ascontiguousarray(w4p[r]),
            "w5p": w5p, "w6p": w6p, "w7p": w7p, "w8p": w8p,
            "wtailp": wtail, "bprep": bpack(r),
        })
    return in_maps


def kernel(**inputs):
    if "nc" not in _CACHE:
        _CACHE["nc"] = _build()
    nc = _CACHE["nc"]
    in_maps = _prep_inputs(inputs)
    trace = bool(int(os.environ.get("KERNEL_TRACE", "0")))
    if trace:
        import ntff_shim
        ntff_shim.install()
    res = run_bass_kernel_spmd(nc, in_maps, core_ids=list(range(NCORES)),
                               trace=trace)
    _CACHE["last_result"] = res
    return res.results[0]["out"]


# revision 8
# speedup vs baseline: 1.7467x; 1.1120x over previous
"""Trainium2 Bass kernel for nn_DomainDiscriminator.

Network: conv(512->256,k3,s3,p1) -> BN -> conv(256->128,k3,s3,p1) -> BN
         -> reshape -> 12-layer MLP (3200->...->1, no nonlinearities) -> sigmoid.
Input x: [64, 512, 40, 40] f32.  Output: [64, 1] f32.

Strategy (8 NeuronCores):
 - Data-parallel batch shard (8 per core) for the convs.
 - stride==kernel==3 convs are non-overlapping patch matmuls. Conv1 patches are
   built host-side (space-to-depth, free); conv2 patches are read straight out
   of SBUF with strided access patterns (boundary-split matmuls, no im2col).
 - Training-mode BN: conv bias is absorbed exactly by BN; BN1 stats via a 2KB
   AllReduce; BN2 stats computed redundantly after an AllGather of the conv2
   raw output.
 - The 12 linear layers have no activations between them, so they compose on
   the host (fp64) into a single [3200] vector + scalar bias; the device does
   one 25-chunk matvec + sigmoid.
 - Convs run in bf16 (BN re-normalizes, keeping error ~2e-3); the final matvec
   in float32r.
"""

import os
import sys

sys.path.insert(0, "/opt/trn_rl_repo")

import numpy as np

import concourse.bass as bass
import concourse.mybir as mybir
import concourse.tile as tile
from concourse import bacc
from concourse.bass_utils import run_bass_kernel_spmd

F32 = mybir.dt.float32
F32R = mybir.dt.float32r
BF16 = mybir.dt.bfloat16

NCORES = 8
BL = 8              # batch per core
B = 64              # full batch
EPS = 1e-5

# conv1: [BL,512,40,40] -> [BL,256,14,14]; conv2: -> [BL,128,5,5]
P1 = 196            # 14*14 positions
P2 = 25             # 5*5 positions
NPT = 4             # conv1 psum tiles (2 batch each)
PTW = 2 * P1        # 392 columns per conv1 psum tile

_CACHE = {}


# ----------------------------------------------------------------------------
# device program
# ----------------------------------------------------------------------------

def _build():
    nc = bacc.Bacc("TRN2", target_bir_lowering=False, debug=False,
                   enable_asserts=True, num_devices=NCORES)

    xprep = nc.dram_tensor("xprep", [4, NPT, 128, 9 * PTW], BF16,
                           kind="ExternalInput")
    w1p = nc.dram_tensor("w1p", [128, 36, 256], BF16, kind="ExternalInput")
    w2p = nc.dram_tensor("w2p", [128, 18, 128], BF16, kind="ExternalInput")
    weffp = nc.dram_tensor("weffp", [128, 25], F32, kind="ExternalInput")
    bprep = nc.dram_tensor("bprep", [128, 7], F32, kind="ExternalInput")
    out = nc.dram_tensor("out", [B, 1], F32, kind="ExternalOutput")
    debug = bool(int(os.environ.get("KERNEL_DEBUG", "0")))
    if debug:
        dbg_h1 = nc.dram_tensor("dbg_h1", [2, 128, 1568], F32, kind="ExternalOutput")
        dbg_g2 = nc.dram_tensor("dbg_g2", [128, 1600], F32, kind="ExternalOutput")
        dbg_st1 = nc.dram_tensor("dbg_st1", [128, 4], F32, kind="ExternalOutput")

    # bprep columns: bn1_g (2), bn1_b (2), bn2_g, bn2_b, beff(row 0)
    BC_BN1G, BC_BN1B, BC_BN2G, BC_BN2B, BC_BEFF = 0, 2, 4, 5, 6

    with tile.TileContext(nc) as tc:
        with tc.tile_pool(name="wp", bufs=1) as wp, \
             tc.tile_pool(name="xp", bufs=4) as xp, \
             tc.tile_pool(name="hp", bufs=1) as hp, \
             tc.tile_pool(name="sp", bufs=1) as sp, \
             tc.tile_pool(name="cps", bufs=4, space="PSUM") as cps, \
             tc.tile_pool(name="c2p", bufs=1, space="PSUM") as c2p, \
             tc.tile_pool(name="zp", bufs=1, space="PSUM") as zp, \
             tc.tile_pool(name="dram", bufs=1, space="DRAM") as dram:

            # ---------------- weight/bias loads (SP ring) ---------------
            w1sb = wp.tile([128, 36 * 256], BF16)
            nc.sync.dma_start(w1sb[:], w1p.ap().rearrange("p a b -> p (a b)"))
            w2sb = wp.tile([128, 18 * 128], BF16)
            nc.sync.dma_start(w2sb[:], w2p.ap().rearrange("p a b -> p (a b)"))
            weff = wp.tile([128, 25], F32)
            nc.sync.dma_start(weff[:], weffp.ap())
            bsb = wp.tile([128, 7], F32)
            nc.sync.dma_start(bsb[:], bprep.ap())

            # ---------------- conv1 -------------------------------------
            h1sb = [hp.tile([128, 4 * PTW], BF16, name=f"h1_{mt}") for mt in range(2)]
            for pt in range(NPT):
                ps = [cps.tile([128, PTW], F32, name="c1ps", tag="c1ps")
                      for _ in range(2)]
                for cb in range(4):
                    xt = xp.tile([128, 9 * PTW], BF16, name="xt", tag="xt")
                    nc.sync.dma_start(xt[:], xprep.ap()[cb, pt])
                    xtr = xt[:].rearrange("p (k c) -> p k c", k=9)
                    for kij in range(9):
                        rhs = xtr[:, kij]
                        for mt in range(2):
                            lhsT = w1sb[:, (cb * 9 + kij) * 256 + mt * 128:
                                        (cb * 9 + kij) * 256 + (mt + 1) * 128]
                            nc.tensor.matmul(ps[mt][:], lhsT, rhs,
                                             start=(cb == 0 and kij == 0),
                                             stop=(cb == 3 and kij == 8))
                for mt in range(2):
                    nc.vector.tensor_copy(
                        h1sb[mt][:, pt * PTW:(pt + 1) * PTW], ps[mt][:])

            # ---------------- BN1 stats + AllReduce ---------------------
            # bounce DMAs ride the Scalar HWDGE ring so they are not stuck
            # behind bulk loads on the SP ring
            scratch = sp.tile([128, 1600], F32)
            st_in = sp.tile([128, 4], F32)
            for mt in range(2):
                h = h1sb[mt][:]
                nc.vector.reduce_sum(st_in[:, 2 * mt:2 * mt + 1], h,
                                     axis=mybir.AxisListType.X)
                nc.scalar.activation(scratch[:, :4 * PTW], h,
                                     mybir.ActivationFunctionType.Square,
                                     accum_out=st_in[:, 2 * mt + 1:2 * mt + 2])
            bn1_in = dram.tile([128, 4], F32)
            bn1_out = dram.tile([128, 4], F32, addr_space="Shared")
            nc.scalar.dma_start(bn1_in[:], st_in[:])
            nc.gpsimd.collective_compute(
                "AllReduce", mybir.AluOpType.add,
                replica_groups=[list(range(NCORES))],
                ins=[bn1_in.opt()], outs=[bn1_out.opt()])
            st1 = sp.tile([128, 4], F32)
            nc.scalar.dma_start(st1[:], bn1_out[:])

            # ---------------- BN1 scale/shift + apply -------------------
            def bn_coeffs(pool, stats_sum, stats_sq, count, g_ap, b_ap, name):
                """returns (scale, shift) [p,1] tiles; stats_* are [p,1] APs"""
                p = stats_sum.shape[0]
                t = pool.tile([p, 6], F32, name=f"bn_{name}")
                mean, msq, vpe, sd, r, tn = (t[:, i:i + 1] for i in range(6))
                nc.vector.tensor_scalar(mean, stats_sum, 1.0 / count, None,
                                        op0=mybir.AluOpType.mult)
                nc.vector.tensor_scalar(vpe, stats_sq, 1.0 / count, None,
                                        op0=mybir.AluOpType.mult)
                nc.vector.tensor_tensor(msq, mean, mean, op=mybir.AluOpType.mult)
                nc.vector.tensor_tensor(vpe, vpe, msq, op=mybir.AluOpType.subtract)
                nc.vector.tensor_scalar(vpe, vpe, EPS, None, op0=mybir.AluOpType.add)
                nc.scalar.activation(sd, vpe, mybir.ActivationFunctionType.Sqrt)
                nc.vector.reciprocal(r, sd)
                # one Newton step: r *= 1.5 - 0.5*vpe*r*r
                nc.vector.tensor_tensor(tn, r, r, op=mybir.AluOpType.mult)
                nc.vector.tensor_tensor(tn, tn, vpe, op=mybir.AluOpType.mult)
                nc.vector.tensor_scalar(tn, tn, -0.5, 1.5,
                                        op0=mybir.AluOpType.mult,
                                        op1=mybir.AluOpType.add)
                nc.vector.tensor_tensor(r, r, tn, op=mybir.AluOpType.mult)
                co = pool.tile([p, 2], F32, name=f"bnc_{name}")
                scale, shift = co[:, 0:1], co[:, 1:2]
                nc.vector.tensor_tensor(scale, g_ap, r, op=mybir.AluOpType.mult)
                nc.vector.tensor_tensor(tn, mean, scale, op=mybir.AluOpType.mult)
                nc.vector.tensor_tensor(shift, b_ap, tn, op=mybir.AluOpType.subtract)
                return scale, shift

            for mt in range(2):
                scale, shift = bn_coeffs(
                    sp, st1[:, 2 * mt:2 * mt + 1], st1[:, 2 * mt + 1:2 * mt + 2],
                    B * P1, bsb[:, BC_BN1G + mt:BC_BN1G + mt + 1],
                    bsb[:, BC_BN1B + mt:BC_BN1B + mt + 1], f"bn1_{mt}")
                nc.vector.tensor_scalar(h1sb[mt][:], h1sb[mt][:],
                                        scale, shift,
                                        op0=mybir.AluOpType.mult,
                                        op1=mybir.AluOpType.add)

            if debug:
                for mt in range(2):
                    dh = sp.tile([128, 1568], F32, name=f"dh{mt}")
                    nc.vector.tensor_copy(dh[:], h1sb[mt][:])
                    nc.sync.dma_start(dbg_h1.ap()[mt], dh[:])
                nc.sync.dma_start(dbg_st1.ap(), st1[:])

            # ---------------- conv2 (strided APs, boundary split) -------
            # psum layout (i2, j2, n): n innermost
            c2ps = c2p.tile([128, P2 * BL], F32)
            c2r = c2ps[:].rearrange("p (i j n) -> p i j n", i=5, j=5, n=BL)
            # full-coverage chunk first so every PSUM element is written once
            kij_order = [(1, 1), (1, 2), (2, 1), (2, 2), (0, 1), (0, 2),
                         (1, 0), (2, 0), (0, 0)]
            nmm = 2 * 9
            cnt = 0
            for cb2 in range(2):
                hr = h1sb[cb2][:].rearrange(
                    "p (n i j) -> p n i j", n=BL, i=14, j=14).transpose([0, 2, 3, 1])
                for (ki, kj) in kij_order:
                    ilo = 1 if ki == 0 else 0
                    jlo = 1 if kj == 0 else 0
                    src = hr[:, 3 * ilo + ki - 1:14:3, 3 * jlo + kj - 1:14:3, :]
                    dst = c2r[:, ilo:, jlo:, :]
                    lhsT = w2sb[:, (cb2 * 9 + ki * 3 + kj) * 128:
                                (cb2 * 9 + ki * 3 + kj + 1) * 128]
                    nc.tensor.matmul(dst, lhsT, src, start=(cnt == 0),
                                     stop=(cnt == nmm - 1), skip_group_check=True)
                    cnt += 1
            c2sb = sp.tile([128, BL * P2], BF16)
            nc.vector.tensor_copy(c2sb[:], c2ps[:])

            # ---------------- AllGather conv2 raw -----------------------
            ag_in = dram.tile([128, BL * P2], BF16)
            ag_out = dram.tile([NCORES, 128, BL * P2], BF16, addr_space="Shared")
            nc.scalar.dma_start(ag_in[:], c2sb[:])
            nc.gpsimd.collective_compute(
                "AllGather", mybir.AluOpType.bypass,
                replica_groups=[list(range(NCORES))],
                ins=[ag_in.opt()], outs=[ag_out.opt()])
            g2 = sp.tile([128, B * P2], BF16)
            nc.scalar.dma_start(
                g2[:].rearrange("p (r t) -> p r t", r=NCORES),
                bass.AP(ag_out.tensor, 0,
                        [[BL * P2, 128], [128 * BL * P2, NCORES], [1, BL * P2]]))

            # ---------------- BN2 (redundant, full batch) ---------------
            st2 = sp.tile([128, 2], F32)
            nc.vector.reduce_sum(st2[:, 0:1], g2[:], axis=mybir.AxisListType.X)
            nc.scalar.activation(scratch[:, :B * P2], g2[:],
                                 mybir.ActivationFunctionType.Square,
                                 accum_out=st2[:, 1:2])
            scale2, shift2 = bn_coeffs(
                sp, st2[:, 0:1], st2[:, 1:2], B * P2,
                bsb[:, BC_BN2G:BC_BN2G + 1], bsb[:, BC_BN2B:BC_BN2B + 1], "bn2")
            g2a = sp.tile([128, B * P2], F32R)
            nc.vector.tensor_scalar(g2a[:], g2[:], scale2, shift2,
                                    op0=mybir.AluOpType.mult,
                                    op1=mybir.AluOpType.add)
            if debug:
                dg = sp.tile([128, 1600], F32, name="dg")
                nc.vector.tensor_copy(dg[:], g2a[:].bitcast(F32))
                nc.sync.dma_start(dbg_g2.ap(), dg[:])

            # ---------------- collapsed MLP: one matvec + sigmoid -------
            # z[n] = sum_{c,ij} weff[c,ij] * g2a[c, (r,ij,n8)] ; out = sigmoid(z + beff)
            weffr = wp.tile([128, 25], F32R)
            nc.vector.tensor_copy(weffr[:], weff[:])
            g2v = g2a[:].rearrange("p (r i n) -> p r i n", r=NCORES, i=P2)
            zps = zp.tile([1, B], F32)
            for ij in range(P2):
                nc.tensor.matmul(zps[:], weffr[:, ij:ij + 1], g2v[:, :, ij, :],
                                 start=(ij == 0), stop=(ij == P2 - 1))
            osb = sp.tile([1, B], F32)
            nc.scalar.activation(osb[:], zps[:],
                                 mybir.ActivationFunctionType.Sigmoid,
                                 bias=bsb[0:1, BC_BEFF:BC_BEFF + 1])
            nc.sync.dma_start(bass.AP(out, 0, [[1, 1], [1, B]]), osb[:])

    nc.compile()
    return nc


# ----------------------------------------------------------------------------
# host-side input prep
# ----------------------------------------------------------------------------

def _prep_inputs(inputs):
    import ml_dtypes
    f = np.float32
    bf = ml_dtypes.bfloat16
    x = np.asarray(inputs["x"], dtype=f)

    # conv1 patches, per core: [4cb, 4pt, 128c, 9kij * 392]
    xpad = np.zeros((B, 512, 42, 42), dtype=bf)
    xpad[:, :, 1:41, 1:41] = x.astype(bf)
    # [n, cb, c, i, ki, j, kj] -> [cb, c, ki, kj, n, i, j]
    xv = xpad.reshape(B, 4, 128, 14, 3, 14, 3).transpose(1, 2, 4, 6, 0, 3, 5)

    w1 = np.asarray(inputs["conv1_w"], dtype=f)          # [256, 512, 3, 3]
    w1p = np.ascontiguousarray(
        w1.reshape(256, 4, 128, 9).transpose(2, 1, 3, 0)).reshape(128, 36, 256).astype(bf)
    w2 = np.asarray(inputs["conv2_w"], dtype=f)          # [128, 256, 3, 3]
    w2p = np.ascontiguousarray(
        w2.reshape(128, 2, 128, 9).transpose(2, 1, 3, 0)).reshape(128, 18, 128).astype(bf)

    # compose the 12 affine layers (no nonlinearities) into [3200] + scalar
    M = np.asarray(inputs["w14"], dtype=np.float64)      # [1, 2]
    beff = np.asarray(inputs["b14"], dtype=np.float64).copy()  # [1]
    for li in range(13, 2, -1):                          # w13 .. w3
        beff += M @ np.asarray(inputs[f"b{li}"], dtype=np.float64)
        M = M @ np.asarray(inputs[f"w{li}"], dtype=np.float64)
    weff = M.reshape(3200).astype(f)                     # order f = c*25 + ij
    weffp = np.ascontiguousarray(weff.reshape(128, 25))
    beff_f = float(beff[0])

    bn1_g = np.asarray(inputs["bn1_g"], dtype=f)
    bn1_b = np.asarray(inputs["bn1_b"], dtype=f)
    bn2_g = np.asarray(inputs["bn2_g"], dtype=f)
    bn2_b = np.asarray(inputs["bn2_b"], dtype=f)

    bp = np.zeros((128, 7), dtype=f)
    bp[:, 0:2] = bn1_g.reshape(2, 128).T
    bp[:, 2:4] = bn1_b.reshape(2, 128).T
    bp[:, 4] = bn2_g
    bp[:, 5] = bn2_b
    bp[0, 6] = beff_f

    in_maps = []
    for r in range(NCORES):
        xr = np.ascontiguousarray(
            xv[:, :, :, :, r * BL:(r + 1) * BL]        # [4,128,3,3,8,14,14]
            .reshape(4, 128, 9, NPT, PTW)
            .transpose(0, 3, 1, 2, 4)                  # [4cb, 4pt, 128, 9, 392]
        ).reshape(4, NPT, 128, 9 * PTW)
        in_maps.append({
            "xprep": xr, "w1p": w1p, "w2p": w2p,
            "weffp": weffp, "bprep": bp,
        })
    return in_maps


def kernel(**inputs):
    if "nc" not in _CACHE:
        _CACHE["nc"] = _build()
    nc = _CACHE["nc"]
    in_maps = _prep_inputs(inputs)
    trace = bool(int(os.environ.get("KERNEL_TRACE", "0")))
    if trace:
        import ntff_shim
        ntff_shim.install()
    res = run_bass_kernel_spmd(nc, in_maps, core_ids=list(range(NCORES)),
                               trace=trace)
    _CACHE["last_result"] = res
    return res.results[0]["out"]


# revision 10
# speedup vs baseline: 2.2849x; 1.3081x over previous
"""Trainium2 Bass kernel for nn_DomainDiscriminator.

Network: conv(512->256,k3,s3,p1) -> BN -> conv(256->128,k3,s3,p1) -> BN
         -> reshape -> 12-layer MLP (3200->...->1, no nonlinearities) -> sigmoid.
Input x: [64, 512, 40, 40] f32.  Output: [64, 1] f32.

Strategy (8 NeuronCores):
 - Data-parallel batch shard (8 per core) for the convs.
 - stride==kernel==3 convs are non-overlapping patch matmuls. Conv1 patches are
   built host-side (space-to-depth, free); conv2 patches are read straight out
   of SBUF with strided access patterns (boundary-split matmuls, no im2col).
 - Training-mode BN: conv bias is absorbed exactly by BN; BN1 stats via a 2KB
   AllReduce; BN2 stats computed redundantly after an AllGather of the conv2
   raw output.
 - The 12 linear layers have no activations between them, so they compose on
   the host (fp64) into a single [3200] vector + scalar bias; the device does
   one 25-chunk matvec + sigmoid.
 - Convs run in bf16 (BN re-normalizes, keeping error ~2e-3); the final matvec
   in float32r.
"""

import os
import sys

sys.path.insert(0, "/opt/trn_rl_repo")

import numpy as np

import concourse.bass as bass
import concourse.mybir as mybir
import concourse.tile as tile
from concourse import bacc
from concourse.bass_utils import run_bass_kernel_spmd

F32 = mybir.dt.float32
F32R = mybir.dt.float32r
BF16 = mybir.dt.bfloat16

NCORES = 8
BL = 8              # batch per core
B = 64              # full batch
EPS = 1e-5

# conv1: [BL,512,40,40] -> [BL,256,14,14]; conv2: -> [BL,128,5,5]
P1 = 196            # 14*14 positions
P2 = 25             # 5*5 positions
NPT = 4             # conv1 psum tiles (2 batch each)
PTW = 2 * P1        # 392 columns per conv1 psum tile

_CACHE = {}


# ----------------------------------------------------------------------------
# device program
# ----------------------------------------------------------------------------

def _build():
    nc = bacc.Bacc("TRN2", target_bir_lowering=False, debug=False,
                   enable_asserts=True, num_devices=NCORES)

    xprep = nc.dram_tensor("xprep", [4, NPT, 128, 9 * PTW], BF16,
                           kind="ExternalInput")
    w1p = nc.dram_tensor("w1p", [128, 36, 256], BF16, kind="ExternalInput")
    w2p = nc.dram_tensor("w2p", [128, 18, 128], BF16, kind="ExternalInput")
    weffp = nc.dram_tensor("weffp", [128, 25], F32, kind="ExternalInput")
    bprep = nc.dram_tensor("bprep", [128, 7], F32, kind="ExternalInput")
    out = nc.dram_tensor("out", [B, 1], F32, kind="ExternalOutput")
    debug = bool(int(os.environ.get("KERNEL_DEBUG", "0")))
    if debug:
        dbg_h1 = nc.dram_tensor("dbg_h1", [2, 128, 1568], F32, kind="ExternalOutput")
        dbg_g2 = nc.dram_tensor("dbg_g2", [128, 1600], F32, kind="ExternalOutput")
        dbg_st1 = nc.dram_tensor("dbg_st1", [128, 4], F32, kind="ExternalOutput")

    # bprep columns: bn1_g (2), bn1_b (2), bn2_g, bn2_b, beff(row 0)
    BC_BN1G, BC_BN1B, BC_BN2G, BC_BN2B, BC_BEFF = 0, 2, 4, 5, 6

    with tile.TileContext(nc) as tc:
        with tc.tile_pool(name="wp", bufs=1) as wp, \
             tc.tile_pool(name="xp", bufs=4) as xp, \
             tc.tile_pool(name="hp", bufs=1) as hp, \
             tc.tile_pool(name="sp", bufs=1) as sp, \
             tc.tile_pool(name="cps", bufs=4, space="PSUM") as cps, \
             tc.tile_pool(name="c2p", bufs=1, space="PSUM") as c2p, \
             tc.tile_pool(name="zp", bufs=1, space="PSUM") as zp, \
             tc.tile_pool(name="dram", bufs=1, space="DRAM") as dram:

            # ---------------- weight/bias loads -------------------------
            w1sb = wp.tile([128, 36 * 256], BF16)
            w1r = w1p.ap().rearrange("p a b -> p (a b)")
            nc.sync.dma_start(w1sb[:, 0:9 * 256], w1r[:, 0:9 * 256])

            # ---------------- conv1 -------------------------------------
            h1sb = [hp.tile([128, 4 * PTW], BF16, name=f"h1_{mt}") for mt in range(2)]
            for pt in range(NPT):
                ps = [cps.tile([128, PTW], F32, name="c1ps", tag="c1ps")
                      for _ in range(2)]
                for cb in range(4):
                    xt = xp.tile([128, 9 * PTW], BF16, name="xt", tag="xt")
                    nc.sync.dma_start(xt[:], xprep.ap()[cb, pt])
                    if pt == 0 and cb < 3:
                        # stream the rest of w1 behind the first x chunk
                        sl = slice((cb + 1) * 9 * 256, (cb + 2) * 9 * 256)
                        nc.sync.dma_start(w1sb[:, sl], w1r[:, sl])
                    xtr = xt[:].rearrange("p (k c) -> p k c", k=9)
                    for kij in range(9):
                        rhs = xtr[:, kij]
                        for mt in range(2):
                            lhsT = w1sb[:, (cb * 9 + kij) * 256 + mt * 128:
                                        (cb * 9 + kij) * 256 + (mt + 1) * 128]
                            nc.tensor.matmul(ps[mt][:], lhsT, rhs,
                                             start=(cb == 0 and kij == 0),
                                             stop=(cb == 3 and kij == 8))
                for mt in range(2):
                    nc.vector.tensor_copy(
                        h1sb[mt][:, pt * PTW:(pt + 1) * PTW], ps[mt][:])

            # late loads (behind the x stream on the SP ring)
            w2sb = wp.tile([128, 18 * 128], BF16)
            nc.sync.dma_start(w2sb[:], w2p.ap().rearrange("p a b -> p (a b)"))
            weff = wp.tile([128, 25], F32)
            nc.sync.dma_start(weff[:], weffp.ap())
            bsb = wp.tile([128, 7], F32)
            nc.sync.dma_start(bsb[:], bprep.ap())

            # ---------------- BN1 stats + AllReduce ---------------------
            # bounce DMAs ride the Scalar HWDGE ring so they are not stuck
            # behind bulk loads on the SP ring
            scratch = sp.tile([128, 1600], F32)
            st_in = sp.tile([128, 4], F32)
            for mt in range(2):
                h = h1sb[mt][:]
                nc.vector.reduce_sum(st_in[:, 2 * mt:2 * mt + 1], h,
                                     axis=mybir.AxisListType.X)
                nc.scalar.activation(scratch[:, :4 * PTW], h,
                                     mybir.ActivationFunctionType.Square,
                                     accum_out=st_in[:, 2 * mt + 1:2 * mt + 2])
            bn1_in = dram.tile([128, 4], F32)
            bn1_out = dram.tile([128, 4], F32, addr_space="Shared")
            nc.scalar.dma_start(bn1_in[:], st_in[:])
            nc.gpsimd.collective_compute(
                "AllReduce", mybir.AluOpType.add,
                replica_groups=[list(range(NCORES))],
                ins=[bn1_in.opt()], outs=[bn1_out.opt()])
            st1 = sp.tile([128, 4], F32)
            nc.scalar.dma_start(st1[:], bn1_out[:])

            # ---------------- BN1 scale/shift + apply -------------------
            def bn_coeffs(pool, stats_sum, stats_sq, count, g_ap, b_ap, name):
                """returns (scale, shift) [p,1] tiles; stats_* are [p,1] APs"""
                p = stats_sum.shape[0]
                t = pool.tile([p, 6], F32, name=f"bn_{name}")
                mean, msq, vpe, sd, r, tn = (t[:, i:i + 1] for i in range(6))
                nc.vector.tensor_scalar(mean, stats_sum, 1.0 / count, None,
                                        op0=mybir.AluOpType.mult)
                nc.vector.tensor_scalar(vpe, stats_sq, 1.0 / count, None,
                                        op0=mybir.AluOpType.mult)
                nc.vector.tensor_tensor(msq, mean, mean, op=mybir.AluOpType.mult)
                nc.vector.tensor_tensor(vpe, vpe, msq, op=mybir.AluOpType.subtract)
                nc.vector.tensor_scalar(vpe, vpe, EPS, None, op0=mybir.AluOpType.add)
                nc.scalar.activation(sd, vpe, mybir.ActivationFunctionType.Sqrt)
                nc.vector.reciprocal(r, sd)
                # one Newton step: r *= 1.5 - 0.5*vpe*r*r
                nc.vector.tensor_tensor(tn, r, r, op=mybir.AluOpType.mult)
                nc.vector.tensor_tensor(tn, tn, vpe, op=mybir.AluOpType.mult)
                nc.vector.tensor_scalar(tn, tn, -0.5, 1.5,
                                        op0=mybir.AluOpType.mult,
                                        op1=mybir.AluOpType.add)
                nc.vector.tensor_tensor(r, r, tn, op=mybir.AluOpType.mult)
                co = pool.tile([p, 2], F32, name=f"bnc_{name}")
                scale, shift = co[:, 0:1], co[:, 1:2]
                nc.vector.tensor_tensor(scale, g_ap, r, op=mybir.AluOpType.mult)
                nc.vector.tensor_tensor(tn, mean, scale, op=mybir.AluOpType.mult)
                nc.vector.tensor_tensor(shift, b_ap, tn, op=mybir.AluOpType.subtract)
                return scale, shift

            for mt in range(2):
                scale, shift = bn_coeffs(
                    sp, st1[:, 2 * mt:2 * mt + 1], st1[:, 2 * mt + 1:2 * mt + 2],
                    B * P1, bsb[:, BC_BN1G + mt:BC_BN1G + mt + 1],
                    bsb[:, BC_BN1B + mt:BC_BN1B + mt + 1], f"bn1_{mt}")
                nc.vector.tensor_scalar(h1sb[mt][:], h1sb[mt][:],
                                        scale, shift,
                                        op0=mybir.AluOpType.mult,
                                        op1=mybir.AluOpType.add)

            if debug:
                for mt in range(2):
                    dh = sp.tile([128, 1568], F32, name=f"dh{mt}")
                    nc.vector.tensor_copy(dh[:], h1sb[mt][:])
                    nc.sync.dma_start(dbg_h1.ap()[mt], dh[:])
                nc.sync.dma_start(dbg_st1.ap(), st1[:])

            # ---------------- conv2 (strided APs, boundary split) -------
            # psum layout (i2, j2, n): n innermost; two parallel psum chains
            # (one per input-channel block), summed by DVE at the end
            kij_order = [(1, 1), (1, 2), (2, 1), (2, 2), (0, 1), (0, 2),
                         (1, 0), (2, 0), (0, 0)]
            c2ps = []
            for cb2 in range(2):
                cp = c2p.tile([128, P2 * BL], F32, name=f"c2ps{cb2}",
                              tag=f"c2ps{cb2}")
                c2ps.append(cp)
                c2r = cp[:].rearrange("p (i j n) -> p i j n", i=5, j=5, n=BL)
                hr = h1sb[cb2][:].rearrange(
                    "p (n i j) -> p n i j", n=BL, i=14, j=14).transpose([0, 2, 3, 1])
                for cnt, (ki, kj) in enumerate(kij_order):
                    ilo = 1 if ki == 0 else 0
                    jlo = 1 if kj == 0 else 0
                    src = hr[:, 3 * ilo + ki - 1:14:3, 3 * jlo + kj - 1:14:3, :]
                    dst = c2r[:, ilo:, jlo:, :]
                    lhsT = w2sb[:, (cb2 * 9 + ki * 3 + kj) * 128:
                                (cb2 * 9 + ki * 3 + kj + 1) * 128]
                    nc.tensor.matmul(dst, lhsT, src, start=(cnt == 0),
                                     stop=(cnt == 8), skip_group_check=True)
            # DVE has a single PSUM read port: go through SBUF for the add
            c2half = sp.tile([128, BL * P2], F32)
            nc.vector.tensor_copy(c2half[:], c2ps[0][:])
            c2sb = sp.tile([128, BL * P2], BF16)
            nc.vector.tensor_tensor(c2sb[:], c2half[:], c2ps[1][:],
                                    op=mybir.AluOpType.add)

            # ---------------- AllGather conv2 raw -----------------------
            ag_in = dram.tile([128, BL * P2], BF16)
            ag_out = dram.tile([NCORES, 128, BL * P2], BF16, addr_space="Shared")
            nc.scalar.dma_start(ag_in[:], c2sb[:])
            nc.gpsimd.collective_compute(
                "AllGather", mybir.AluOpType.bypass,
                replica_groups=[list(range(NCORES))],
                ins=[ag_in.opt()], outs=[ag_out.opt()])
            g2 = sp.tile([128, B * P2], BF16)
            nc.scalar.dma_start(
                g2[:].rearrange("p (r t) -> p r t", r=NCORES),
                bass.AP(ag_out.tensor, 0,
                        [[BL * P2, 128], [128 * BL * P2, NCORES], [1, BL * P2]]))

            # ---------------- BN2 (redundant, full batch) ---------------
            st2 = sp.tile([128, 2], F32)
            nc.vector.reduce_sum(st2[:, 0:1], g2[:], axis=mybir.AxisListType.X)
            nc.scalar.activation(scratch[:, :B * P2], g2[:],
                                 mybir.ActivationFunctionType.Square,
                                 accum_out=st2[:, 1:2])
            scale2, shift2 = bn_coeffs(
                sp, st2[:, 0:1], st2[:, 1:2], B * P2,
                bsb[:, BC_BN2G:BC_BN2G + 1], bsb[:, BC_BN2B:BC_BN2B + 1], "bn2")
            g2a = sp.tile([128, B * P2], F32R)
            nc.vector.tensor_scalar(g2a[:], g2[:], scale2, shift2,
                                    op0=mybir.AluOpType.mult,
                                    op1=mybir.AluOpType.add)
            if debug:
                dg = sp.tile([128, 1600], F32, name="dg")
                nc.vector.tensor_copy(dg[:], g2a[:].bitcast(F32))
                nc.sync.dma_start(dbg_g2.ap(), dg[:])

            # ---------------- collapsed MLP: one matvec + sigmoid -------
            # z[n] = sum_{c,ij} weff[c,ij] * g2a[c, (r,ij,n8)] ; out = sigmoid(z + beff)
            weffr = wp.tile([128, 25], F32R)
            nc.vector.tensor_copy(weffr[:], weff[:])
            g2v = g2a[:].rearrange("p (r i n) -> p r i n", r=NCORES, i=P2)
            zps = zp.tile([1, B], F32)
            for ij in range(P2):
                nc.tensor.matmul(zps[:], weffr[:, ij:ij + 1], g2v[:, :, ij, :],
                                 start=(ij == 0), stop=(ij == P2 - 1))
            osb = sp.tile([1, B], F32)
            nc.scalar.activation(osb[:], zps[:],
                                 mybir.ActivationFunctionType.Sigmoid,
                                 bias=bsb[0:1, BC_BEFF:BC_BEFF + 1])
            nc.sync.dma_start(bass.AP(out, 0, [[1, 1], [1, B]]), osb[:])

    nc.compile()
    return nc


# ----------------------------------------------------------------------------
# host-side input prep
# ----------------------------------------------------------------------------

def _prep_inputs(inputs):
    import ml_dtypes
    f = np.float32
    bf = ml_dtypes.bfloat16
    x = np.asarray(inputs["x"], dtype=f)

    # conv1 patches, per core: [4cb, 4pt, 128c, 9kij * 392]
    xpad = np.zeros((B, 512, 42, 42), dtype=bf)
    xpad[:, :, 1:41, 1:41] = x.astype(bf)
    # [n, cb, c, i, ki, j, kj] -> [cb, c, ki, kj, n, i, j]
    xv = xpad.reshape(B, 4, 128, 14, 3, 14, 3).transpose(1, 2, 4, 6, 0, 3, 5)

    w1 = np.asarray(inputs["conv1_w"], dtype=f)          # [256, 512, 3, 3]
    w1p = np.ascontiguousarray(
        w1.reshape(256, 4, 128, 9).transpose(2, 1, 3, 0)).reshape(128, 36, 256).astype(bf)
    w2 = np.asarray(inputs["conv2_w"], dtype=f)          # [128, 256, 3, 3]
    w2p = np.ascontiguousarray(
        w2.reshape(128, 2, 128, 9).transpose(2, 1, 3, 0)).reshape(128, 18, 128).astype(bf)

    # compose the 12 affine layers (no nonlinearities) into [3200] + scalar
    M = np.asarray(inputs["w14"], dtype=np.float64)      # [1, 2]
    beff = np.asarray(inputs["b14"], dtype=np.float64).copy()  # [1]
    for li in range(13, 2, -1):                          # w13 .. w3
        beff += M @ np.asarray(inputs[f"b{li}"], dtype=np.float64)
        M = M @ np.asarray(inputs[f"w{li}"], dtype=np.float64)
    weff = M.reshape(3200).astype(f)                     # order f = c*25 + ij
    weffp = np.ascontiguousarray(weff.reshape(128, 25))
    beff_f = float(beff[0])

    bn1_g = np.asarray(inputs["bn1_g"], dtype=f)
    bn1_b = np.asarray(inputs["bn1_b"], dtype=f)
    bn2_g = np.asarray(inputs["bn2_g"], dtype=f)
    bn2_b = np.asarray(inputs["bn2_b"], dtype=f)

    bp = np.zeros((128, 7), dtype=f)
    bp[:, 0:2] = bn1_g.reshape(2, 128).T
    bp[:, 2:4] = bn1_b.reshape(2, 128).T
    bp[:, 4] = bn2_g
    bp[:, 5] = bn2_b
    bp[0, 6] = beff_f

    in_maps = []
    for r in range(NCORES):
        xr = np.ascontiguousarray(
            xv[:, :, :, :, r * BL:(r + 1) * BL]        # [4,128,3,3,8,14,14]
            .reshape(4, 128, 9, NPT, PTW)
            .transpose(0, 3, 1, 2, 4)                  # [4cb, 4pt, 128, 9, 392]
        ).reshape(4, NPT, 128, 9 * PTW)
        in_maps.append({
            "xprep": xr, "w1p": w1p, "w2p": w2p,
            "weffp": weffp, "bprep": bp,
        })
    return in_maps


def kernel(**inputs):
    if "nc" not in _CACHE:
        _CACHE["nc"] = _build()
    nc = _CACHE["nc"]
    in_maps = _prep_inputs(inputs)
    trace = bool(int(os.environ.get("KERNEL_TRACE", "0")))
    if trace:
        import ntff_shim
        ntff_shim.install()
    res = run_bass_kernel_spmd(nc, in_maps, core_ids=list(range(NCORES)),
                               trace=trace)
    _CACHE["last_result"] = res
    return res.results[0]["out"]
